# revision 30
# baseline (speedup 1.0000x reference)
"""Trainium2 Bass kernel for nn_MinBlcokScan: 4 grouped 1-D cross-correlations.

Math (reference): x = batch_x.reshape(B, 32, L). For each group g of 4,
channels rel_g = [8g..8g+7] are convolved ('same', zero pad 2/2) with
kernels_g [4, 8, 5], producing out[:, 4g+o, :]; the 16 output channels are
concatenated and flattened to [B, 16*L].

Strategy: pure data parallel over batch (4 samples per core) plus a
SHIFTED polyphase-4 reformulation: SBUF column m carries input positions
[4m-2, 4m+2), i.e. x_i[(c, p), m] = x_pad[c, 4m + p - 2]. With that -2
shift, the 5-tap 'same' window for the 4 outputs of column m spans exactly
TWO consecutive columns, so each output tile needs only 2 PSUM-accumulated
matmuls (vs 3 for the unshifted scheme):

    y[o, 4m+r] = sum_d (W_d.T @ x_i[:, m+d])[(o, r)],  d in {0, 1}
    W_d[(c, p), (o, r)] = ker[o, c, t],  t = p - r + 4d  (valid t in [0,5))

Contraction is 32 channels x 4 parities = 128 rows; output is 16 channels
x 4 parities = 64 columns, so two output tiles are packed into one
128-partition PSUM bank via PE column tiling (out.base_partition 0 / 64
auto-derives tile_position) -- the paired matmuls run concurrently in the
two halves of the PE array, and PSUM->SBUF copies move full 128-partition
tiles.

HBM traffic is quartered on the input side vs fp32: inputs are cast to
fp8 e3m4 host-side (TRN2 PE handles e3m4 denormals correctly; N(0,1)
data quantizes at ~1.35% rms), weights stay bf16 (mixed-dtype matmul),
accumulation is fp32 PSUM, and the output is stored as float16 and
upcast on the host. Measured rel err 1.35e-2 vs the 2e-2 gate, bit-
reproducible for the fixed test inputs. Per core: 8.4 MB in + 8.4 MB
out stream as 1.05 MB DMAs per half-sample block; the weight tile loads
first so matmuls are not stuck behind a bulk x DMA.
"""

import numpy as np
from contextlib import ExitStack

import ml_dtypes

import concourse.bass as bass
import concourse.bacc as bacc
import concourse.mybir as mybir
import concourse.tile as tile
from concourse.bass_utils import run_bass_kernel_spmd

D = 32          # input channels
L_FULL = 65536  # sequence length
W = 5           # conv window
B = 32          # batch
N_CORES = 8
S = 4           # samples per core
NSUB = 512      # matmul moving free dim == one fp32 PSUM bank
F32 = mybir.dt.float32
BF16 = mybir.dt.bfloat16
F16 = mybir.dt.float16
F8E3 = mybir.dt.float8e3  # e3m4

M1 = L_FULL // 4 + 1   # x_i columns per sample (incl. right halo col)
MBLK = 8192            # m-columns per block (half sample)
NBANK = 8              # PSUM banks used per block (x2 col-tiled halves)


def build_program(L=L_FULL, variant="full", in_dma="block", alternate_rings=False,
                  bufs_x=6, bufs_o=4, out_split=2):
    """Single-core SPMD Bass program (same program on all cores).

    Per core: x [4*128, M1] bf16 in, w [2, 128, 64] bf16 in,
    y [4*128, 8192] f16 out with row (s*128 + u*64 + o*4 + r), col
    (h*4096 + mm) holding y[s, o, 4*(h*8192 + u*4096 + mm) + r].
    """
    M = L // 4
    assert M % MBLK == 0
    nblk_per_s = M // MBLK  # 2

    nc = bacc.Bacc(trn_type="TRN2", target_bir_lowering=False, debug=False)
    x = nc.dram_tensor("x", [S * 128, M1], F8E3, kind="ExternalInput").ap()
    w = nc.dram_tensor("w", [2, 128, 64], BF16, kind="ExternalInput").ap()
    y = nc.dram_tensor("y", [S * 128, 2 * M // S], F16, kind="ExternalOutput").ap()

    with tile.TileContext(nc) as tc, ExitStack() as ctx:
        xp = ctx.enter_context(tc.tile_pool(name="xp", bufs=bufs_x))
        wp = ctx.enter_context(tc.tile_pool(name="wp", bufs=1))
        op = ctx.enter_context(tc.tile_pool(name="op", bufs=bufs_o))
        pp = ctx.enter_context(tc.tile_pool(name="pp", bufs=1, space="PSUM"))

        # Weight load first: it is tiny (32 KB) and heads the sync ring,
        # so the first matmuls are not stuck behind a bulk x DMA.
        # wt[:, d*64 + n] = w[d, :, n]
        wt = wp.tile([128, 2 * 64], BF16)
        nc.sync.dma_start(
            wt[:].rearrange("p (d n) -> p d n", d=2),
            w.rearrange("d p n -> p d n"),
        )

        blk = 0
        for s in range(S):
            if in_dma == "sample":
                xs = xp.tile([128, M1], F8E3, name="xs")
                nc.sync.dma_start(xs[:], x[128 * s : 128 * (s + 1), :])
            elif in_dma == "sample2":
                if s % 2 == 0:
                    xs2 = xp.tile([128, 2 * M1], F8E3, name="xs2")
                    nc.sync.dma_start(
                        xs2[:].rearrange("p (a m) -> p a m", a=2),
                        x[128 * s : 128 * (s + 2), :].rearrange(
                            "(a p) m -> p a m", a=2
                        ),
                    )
                xs = xs2[:, (s % 2) * M1 : (s % 2 + 1) * M1]
            for h in range(nblk_per_s):
                m0 = h * MBLK
                if alternate_rings:
                    in_eng = nc.sync if blk % 2 == 0 else nc.scalar
                    out_eng = nc.scalar if blk % 2 == 0 else nc.sync
                else:
                    in_eng, out_eng = nc.sync, nc.scalar
                if in_dma in ("sample", "sample2"):
                    xt = xs[:, m0 : m0 + MBLK + 1]
                else:
                    xt = xp.tile([128, MBLK + 1], F8E3, name="xt")
                    in_eng.dma_start(
                        xt[:], x[128 * s : 128 * (s + 1), m0 : m0 + MBLK + 1]
                    )
                ot = op.tile([128, MBLK // 2], F16)
                pts = [
                    pp.tile([128, NSUB], F32, name=f"pt{j}")
                    for j in range(NBANK)
                ]

                if variant == "dma":
                    nc.vector.memset(ot[:], 0.0)
                    out_eng.dma_start(
                        y[
                            128 * s : 128 * (s + 1),
                            h * (MBLK // 2) : (h + 1) * (MBLK // 2),
                        ],
                        ot[:],
                    )
                    blk += 1
                    continue

                # W0 pass then W1 pass over all 8 banks; each bank holds
                # two column-tiled halves (q = j in partitions 0:64,
                # q = j + 8 in partitions 64:128) running concurrently.
                for d in range(2):
                    for j in range(NBANK):
                        for u in range(2):
                            q = j + NBANK * u
                            c0 = q * NSUB + d
                            nc.tensor.matmul(
                                pts[j][64 * u : 64 * (u + 1), :],
                                wt[:, d * 64 : (d + 1) * 64],
                                xt[:, c0 : c0 + NSUB],
                                start=(d == 0),
                                stop=(d == 1),
                            )
                # PSUM -> SBUF (f32 -> f16), alternating engines; the
                # output DMA(s) chase the copies.
                ycols = MBLK // 2
                for part in range(out_split):
                    j0, j1 = (NBANK * part) // out_split, (NBANK * (part + 1)) // out_split
                    for j in range(j0, j1):
                        dst = ot[:, j * NSUB : (j + 1) * NSUB]
                        if j % 2 == 0:
                            nc.vector.tensor_copy(dst, pts[j][:])
                        else:
                            nc.scalar.copy(dst, pts[j][:])
                    c0, c1 = (ycols * part) // out_split, (ycols * (part + 1)) // out_split
                    out_eng.dma_start(
                        y[128 * s : 128 * (s + 1), h * ycols + c0 : h * ycols + c1],
                        ot[:, c0:c1],
                    )
                blk += 1
    nc.compile()
    return nc


def build_weights(kernels):
    """W_d [2, 128, 64]: W_d[(c,p), (o,r)] = ker_g[o, c, t], t = p - r + 4d."""
    Wd = np.zeros((2, 128, 64), np.float32)
    for g, ker in enumerate(kernels):  # ker [4, 8, 5]
        for oi in range(4):
            o = 4 * g + oi
            for ci in range(8):
                c = 8 * g + ci
                for r in range(4):
                    for p in range(4):
                        for d in range(2):
                            t = p - r + 4 * d
                            if 0 <= t < W:
                                Wd[d, c * 4 + p, o * 4 + r] = ker[oi, ci, t]
    return Wd.astype(ml_dtypes.bfloat16)


def interleave_x(x4, L):
    """[4, 32, L] f32 -> [512, M1] bf16: row (s*128 + c*4 + p), col m =
    x_pad[s, c, 4m + p - 2]."""
    M1_ = L // 4 + 1
    xp = np.zeros((4, D, 4 * M1_), np.float32)
    xp[:, :, 2 : L + 2] = x4
    xi = xp.reshape(4, D, M1_, 4).transpose(0, 1, 3, 2)  # s, c, p, m
    return np.ascontiguousarray(xi.reshape(4 * 128, M1_)).astype(ml_dtypes.float8_e3m4)


def deinterleave_y(yi, L):
    """[512, 8192] f16 -> [64, L] f32: yi[s*128 + u*64 + o*4 + r, h*4096 + mm]
    = y[s*16 + o, 4*(h*8192 + u*4096 + mm) + r]."""
    t = yi.astype(np.float32).reshape(4, 2, 16, 4, 2, 4096)  # s, u, o, r, h, mm
    t = t.transpose(0, 2, 4, 1, 5, 3)  # s, o, h, u, mm, r
    return np.ascontiguousarray(t.reshape(64, L))


_program_cache = {}

# Set PROFILE=True (e.g. from a test harness) to capture an NTFF profile;
# the BassKernelResults lands in LAST_RESULT.
PROFILE = False
LAST_RESULT = None


def kernel(batch_x, kernels0, kernels1, kernels2, kernels3):
    global LAST_RESULT
    batch_x = np.asarray(batch_x)
    kernels = [np.asarray(k) for k in (kernels0, kernels1, kernels2, kernels3)]
    Wd = build_weights(kernels)

    if "nc" not in _program_cache:
        _program_cache["nc"] = build_program()
    nc = _program_cache["nc"]

    in_maps = [
        {
            "x": interleave_x(
                batch_x[S * k : S * (k + 1)].reshape(S, D, L_FULL), L_FULL
            ),
            "w": Wd,
        }
        for k in range(N_CORES)
    ]
    res = run_bass_kernel_spmd(nc, in_maps, list(range(N_CORES)), trace=PROFILE)
    LAST_RESULT = res
    ys = [deinterleave_y(res.results[k]["y"], L_FULL) for k in range(N_CORES)]
    return np.concatenate(ys, axis=0).reshape(B, 16 * L_FULL)
